# revision 2
# baseline (speedup 1.0000x reference)
"""GQA attention (B=4,S=1024,D=2048,H=32,KVH=8,HD=64) + RoPE, tensor-parallel
over the 8 kv-head groups across 8 NeuronCores.

v2: single interleaved instruction stream keeps the PE dense (HAM stays at
8/8) and overlaps the ACT exp stream with projection/AV/output matmuls.

Per-core pipeline (transposed layouts):
  qT/kT/vT = W.T @ xT          (PE, bf16, D-contraction in 16 chunks of 128)
  RoPE: qrot = (q*cos) + A@(q*sin)
  sT[ev|od] = kT.T-block @ qT  (two K=64 MMs in disjoint PE row groups -> run
                                concurrently; ev/od = even/odd head of pair)
  pT = exp(sT/8)               (ACT, [128,1024] slabs)
  av_* = [v|1].T @ p_*         (K=128; denom in row 64; ev/od separate banks)
  avn = av * bcast(1/denom)    (DVE approx-recip + gpsimd partition_broadcast;
                                heads stacked on 128 partitions via DVE shift)
  y += avn.T-chunk @ Wo_pair   (K=128 over stacked head pair), bf16 out,
                               host sums the 8 partial y's.
"""

import numpy as np
import ml_dtypes

import concourse.bass as bass
import concourse.mybir as mybir
import concourse.tile as tile
from concourse import bacc
from concourse import bass_utils

BF16 = mybir.dt.bfloat16
F32 = mybir.dt.float32
BF = ml_dtypes.bfloat16

B, S, D = 4, 1024, 2048
H, KVH, HD = 32, 8, 64
NREP = H // KVH          # 4 q heads per core (2 pairs)
T = B * S                # 4096 tokens
NC = 8                   # cores
QD = NREP * HD           # 256 q dims per core
KC = D // 128            # 16 contraction chunks
TB = 512                 # proj token-block
NTB = T // TB            # 8
VB = 66                  # v_aug block stride: [v*64, 1, pad]
AF = mybir.ActivationFunctionType

# broadcast method: "dram" (gpsimd dma via DRAM roundtrip, known-good) or
# "pbcast" (gpsimd partition_broadcast from a partition-0 source row).
BCAST = "dram"

_CACHE = {}


def _build(debug=False):
    key = ("nc", debug)
    if key in _CACHE:
        return _CACHE[key]
    nc = bacc.Bacc("TRN2", target_bir_lowering=False)
    # Pin all ACT table lookups to set 6 (natural_log_exp_and_others) so the
    # kernel needs exactly one table load.
    import concourse.bacc as _bacc_mod
    _orig_tables = _bacc_mod.get_activation_tables

    def _pinned_tables(arch):
        items = list(_orig_tables(arch).items())
        return {k: (v if i == 6 else set()) for i, (k, v) in enumerate(items)}

    _bacc_mod.get_activation_tables = _pinned_tables

    xT_d = nc.dram_tensor("xT", (D, T), BF16, kind="ExternalInput")
    wq_d = nc.dram_tensor("wq", (D, QD), BF16, kind="ExternalInput")
    wkv_d = nc.dram_tensor("wkv", (D, 128), BF16, kind="ExternalInput")
    wo_d = nc.dram_tensor("wo", (QD, D), BF16, kind="ExternalInput")
    cos_d = nc.dram_tensor("cos2", (128, S), F32, kind="ExternalInput")
    sin_d = nc.dram_tensor("sin2", (128, S), F32, kind="ExternalInput")
    arot_d = nc.dram_tensor("arot", (128, 128), BF16, kind="ExternalInput")
    eye_d = nc.dram_tensor("eye64", (64, 64), BF16, kind="ExternalInput")
    y_d = nc.dram_tensor("y", (T, D), BF16, kind="ExternalOutput")
    dbg = {}
    if debug:
        for nm, shape, dt in [
            ("dump_q", (128, T), BF16), ("dump_k", (128, T), BF16),
            ("dump_vaug", (128, 8 * VB), BF16), ("dump_prob", (128, 16 * 512), BF16),
            ("dump_avn", (128, S), BF16), ("dump_rbc", (128, 512), F32),
        ]:
            dbg[nm] = nc.dram_tensor(nm, shape, dt, kind="ExternalOutput")

    with tile.TileContext(nc) as tc:
        with (
            tc.tile_pool(name="const", bufs=1) as cpool,
            tc.tile_pool(name="persist", bufs=1) as ppool,
            tc.tile_pool(name="xin", bufs=2) as xpool,
            tc.tile_pool(name="rtmp", bufs=2) as rpool,
            tc.tile_pool(name="prob", bufs=3) as prpool,
            tc.tile_pool(name="nrm", bufs=2) as npool,
            tc.tile_pool(name="avns", bufs=4) as apool,
            tc.tile_pool(name="yout", bufs=2) as ypool,
            tc.tile_pool(name="pslab", bufs=2, space="PSUM") as pslab,
            tc.tile_pool(name="pav", bufs=1, space="PSUM") as pav,
            tc.tile_pool(name="ppj", bufs=1, space="PSUM") as ppj,
            tc.tile_pool(name="pm", bufs=1, space="PSUM") as pm,
            tc.tile_pool(name="dscr", bufs=2, space="DRAM") as dpool,
        ):
            # ---- constants ----
            wq_sb = cpool.tile([128, KC * QD], BF16, tag="wq")
            wq_dv = wq_d[:].rearrange("(c p) m -> p c m", p=128)
            wq_sv = wq_sb[:].rearrange("p (c m) -> p c m", c=KC)
            nc.sync.dma_start(out=wq_sv[:, 0:4, :], in_=wq_dv[:, 0:4, :])
            wkv_sb = cpool.tile([128, KC * 128], BF16, tag="wkv")
            nc.sync.dma_start(
                out=wkv_sb[:].rearrange("p (c m) -> p c m", c=KC),
                in_=wkv_d[:].rearrange("(c p) m -> p c m", p=128),
            )
            nc.sync.dma_start(out=wq_sv[:, 4:KC, :], in_=wq_dv[:, 4:KC, :])
            # Wo as two stacked head-pair tiles [128, D]
            wo_sb = [cpool.tile([128, D], BF16, tag=f"wo{p}", name=f"wo{p}") for p in range(2)]
            nc.sync.dma_start(out=wo_sb[0][:], in_=wo_d[0:128, :])
            nc.sync.dma_start(out=wo_sb[1][:], in_=wo_d[128:256, :])
            cos_sb = cpool.tile([128, S], F32, tag="cos")
            nc.sync.dma_start(out=cos_sb[:], in_=cos_d[:])
            sin_sb = cpool.tile([128, S], F32, tag="sin")
            nc.sync.dma_start(out=sin_sb[:], in_=sin_d[:])
            arot_sb = cpool.tile([128, 128], BF16, tag="arot")
            nc.sync.dma_start(out=arot_sb[:], in_=arot_d[:])
            eye_sb = cpool.tile([64, 64], BF16, tag="eye")
            nc.sync.dma_start(out=eye_sb[:], in_=eye_d[:])

            # ---- persistent activations ----
            qrope = [ppool.tile([128, T], BF16, tag=f"qrope{p}", name=f"qrope{p}") for p in range(2)]
            kT2 = ppool.tile([128, T], BF16, tag="kT2")
            vtmpT = ppool.tile([64, T], BF16, tag="vtmpT")
            v_aug = [ppool.tile([128, 8 * VB], BF16, tag=f"vaug{b}", name=f"vaug{b}") for b in range(B)]
            avn = {}
            probs = {}
            xts_tiles = {}

            def xts_load(tb):
                xts = xpool.tile([128, KC * TB], BF16, tag="xts", name=f"xts{tb}")
                xv = xts[:].rearrange("p (c n) -> p c n", c=KC)
                dv = xT_d[:, bass.ts(tb, TB)].rearrange("(c p) n -> p c n", p=128)
                # split by chunk group so the first matmuls start early
                for c0 in range(0, KC, 4):
                    nc.sync.dma_start(out=xv[:, c0:c0 + 4, :], in_=dv[:, c0:c0 + 4, :])
                xts_tiles[tb] = xts

            xts_load(0)
            xts_load(1)

            # ================= work generators =================
            def gen_proj(tb):
                b, scol = tb // 2, (tb % 2) * TB
                tcols = bass.ts(tb, TB)
                xts = xts_tiles.pop(tb)
                if tb + 2 < NTB:
                    xts_load(tb + 2)
                css, sns = cos_sb[:, scol:scol + TB], sin_sb[:, scol:scol + TB]
                # q pieces then kv piece; each accumulates on the pj bank
                for piece in range(3):
                    qp = ppj.tile([128, TB], F32, tag="pj", name=f"pj{tb}_{piece}")
                    for c0 in range(0, KC, 4):
                        for c in range(c0, c0 + 4):
                            if piece < 2:
                                w = wq_sb[:, c * QD + piece * 128: c * QD + (piece + 1) * 128]
                            else:
                                w = wkv_sb[:, bass.ts(c, 128)]
                            nc.tensor.matmul(qp[:], w, xts[:, bass.ts(c, TB)],
                                             start=(c == 0), stop=(c == KC - 1))
                        yield
                    if piece < 2:
                        qsin = rpool.tile([128, TB], BF16, tag="qsin")
                        nc.vector.tensor_mul(qsin[:], qp[:], sns)
                        t1 = rpool.tile([128, TB], F32, tag="t1")
                        nc.vector.tensor_mul(t1[:], qp[:], css)
                        sh = pm.tile([128, TB], F32, tag="m", name=f"shq{tb}_{piece}")
                        nc.tensor.matmul(sh[:], arot_sb[:], qsin[:], start=True, stop=True)
                        nc.vector.tensor_add(qrope[piece][:, tcols], t1[:], sh[:])
                        yield
                    else:
                        ksin = rpool.tile([64, TB], BF16, tag="ksin")
                        nc.vector.tensor_mul(ksin[:], qp[0:64, :], sns[0:64])
                        t1k = rpool.tile([64, TB], F32, tag="t1k")
                        nc.vector.tensor_mul(t1k[:], qp[0:64, :], css[0:64])
                        shk = pm.tile([128, TB], F32, tag="m", name=f"shk{tb}")
                        nc.tensor.matmul(shk[0:64, :], arot_sb[0:64, 0:64], ksin[:],
                                         start=True, stop=True)
                        nc.vector.tensor_add(kT2[0:64, tcols], t1k[:], shk[0:64, :])
                        # duplicate k rows for the odd-head row group
                        nc.sync.dma_start(out=kT2[64:128, tcols], in_=kT2[0:64, tcols])
                        # v: psum rows 64:128 -> vtmpT partitions 0:64 (DVE shift)
                        nc.vector.tensor_copy(vtmpT[:, tcols], qp[64:128, :])
                        yield
                if tb % 2 == 1:
                    # v_aug build for batch b: per kb block [v(64) | 1 | pad]
                    va = v_aug[b]
                    va3 = va[:].rearrange("p (k c) -> p k c", k=8)
                    nc.vector.memset(va[:], 0.0)
                    nc.vector.memset(va3[:, :, 64:65], 1.0)
                    vslab = pm.tile([128, 512], BF16, tag="m", name=f"vslab{b}")
                    for kb in range(8):
                        nc.tensor.transpose(
                            vslab[:, kb * 64:(kb + 1) * 64],
                            vtmpT[:, b * S + kb * 128: b * S + (kb + 1) * 128],
                            eye_sb[:],
                        )
                    yield
                    nc.vector.tensor_copy(
                        va3[:, :, 0:64],
                        vslab[:].rearrange("p (k c) -> p k c", k=8),
                    )
                    yield

            def gen_S(u):
                b, pr = u // 2, u % 2
                for qh in range(2):
                    prob = prpool.tile([128, 8192], BF16, tag="prob", name=f"prob{u}_{qh}")
                    probs[(u, qh)] = prob
                    for kb in range(8):
                        sl = pslab.tile([128, 1024], F32, tag="s", name=f"sl{u}_{qh}_{kb}")
                        qc = slice(b * S + qh * 512, b * S + (qh + 1) * 512)
                        kc = slice(b * S + kb * 128, b * S + (kb + 1) * 128)
                        nc.tensor.matmul(sl[:, 0:512], kT2[0:64, kc], qrope[pr][0:64, qc],
                                         start=True, stop=True, tile_position=(0, 0))
                        nc.tensor.matmul(sl[:, 512:1024], kT2[64:128, kc], qrope[pr][64:128, qc],
                                         start=True, stop=True, tile_position=(64, 0))
                        nc.scalar.activation(prob[:, bass.ts(kb, 1024)], sl[:],
                                             AF.Exp, scale=0.125)
                        yield
                    if debug and u == 0 and qh == 1:
                        nc.sync.dma_start(out=dbg["dump_prob"][:], in_=prob[:])

            def gen_AV(u):
                b, pr = u // 2, u % 2
                va = v_aug[b]
                avn_t = apool.tile([128, S], BF16, tag="avn", name=f"avn{u}")
                avn[u] = avn_t
                for qh in range(2):
                    prob = probs.pop((u, qh))
                    ae = pav.tile([128, 512], F32, tag="ae", name=f"ae{u}_{qh}")
                    ao = pav.tile([128, 512], F32, tag="ao", name=f"ao{u}_{qh}")
                    for kb in range(8):
                        st = dict(start=(kb == 0), stop=(kb == 7))
                        va_k = va[:, kb * VB: kb * VB + 65]
                        nc.tensor.matmul(ae[0:65, :], va_k,
                                         prob[:, kb * 1024: kb * 1024 + 512], **st)
                        nc.tensor.matmul(ao[0:65, :], va_k,
                                         prob[:, kb * 1024 + 512: (kb + 1) * 1024], **st)
                        if kb % 4 == 3:
                            yield
                    # den rows -> partition-0 tiles via ACT (honors offsets),
                    # then custom-DVE approx recip + partition_broadcast, both
                    # of which require tile-base (partition-0) APs.
                    cpe = npool.tile([1, 512], F32, tag="cpe", name=f"cpe{u}_{qh}")
                    nc.scalar.copy(cpe[:], ae[64:65, :])
                    cpo = npool.tile([1, 512], F32, tag="cpo", name=f"cpo{u}_{qh}")
                    nc.scalar.copy(cpo[:], ao[64:65, :])
                    dne = npool.tile([1, 512], F32, tag="dne", name=f"dne{u}_{qh}")
                    nc.vector.reciprocal_approx_fast(dne[:], cpe[:])
                    dno = npool.tile([1, 512], F32, tag="dno", name=f"dno{u}_{qh}")
                    nc.vector.reciprocal_approx_fast(dno[:], cpo[:])
                    rbc = npool.tile([64, 512], F32, tag="rbc", name=f"rbc{u}_{qh}")
                    nc.gpsimd.partition_broadcast(rbc[:], dne[:], channels=64)
                    rbco = npool.tile([64, 512], F32, tag="rbco", name=f"rbco{u}_{qh}")
                    nc.gpsimd.partition_broadcast(rbco[:], dno[:], channels=64)
                    yield
                    qcols = bass.ts(qh, 512)
                    nc.vector.tensor_mul(avn_t[0:64, qcols], ae[0:64, :], rbc[:])
                    tmpo = npool.tile([64, 512], BF16, tag="tmpo", name=f"tmpo{u}_{qh}")
                    nc.vector.tensor_mul(tmpo[:], ao[0:64, :], rbco[:])
                    nc.vector.tensor_copy(avn_t[64:128, qcols], tmpo[:])
                    yield
                    if debug and u == 0 and qh == 1:
                        nc.sync.dma_start(out=dbg["dump_avn"][:], in_=avn_t[:])
                        nc.sync.dma_start(out=dbg["dump_rbc"][0:64, :], in_=rbc[:])
                        nc.sync.dma_start(out=dbg["dump_rbc"][64:128, :], in_=rbco[:])

            def gen_Y(b, delay=0):
                for _ in range(delay):
                    yield
                for t in range(8):
                    ysb = ypool.tile([128, D], BF16, tag="ysb", name=f"ysb{b}_{t}")
                    for nbp in range(2):
                        ysl = pslab.tile([128, 1024], F32, tag="s", name=f"ysl{b}_{t}_{nbp}")
                        for half in range(2):
                            nb = nbp * 2 + half
                            for pr in range(2):
                                nc.tensor.matmul(
                                    ysl[:, bass.ts(half, 512)],
                                    avn[b * 2 + pr][:, bass.ts(t, 128)],
                                    wo_sb[pr][:, bass.ts(nb, 512)],
                                    start=(pr == 0), stop=(pr == 1),
                                )
                        nc.vector.tensor_copy(ysb[:, bass.ts(nbp, 1024)], ysl[:])
                        yield
                    nc.sync.dma_start(
                        out=y_d[b * S + t * 128: b * S + (t + 1) * 128, :], in_=ysb[:])

            def run_phase(gens):
                gens = [g for g in gens if g is not None]
                while gens:
                    nxt = []
                    for g in gens:
                        try:
                            next(g)
                            nxt.append(g)
                        except StopIteration:
                            pass
                    gens = nxt

            # ================= schedule =================
            run_phase([gen_proj(0)])
            run_phase([gen_proj(1)])
            run_phase([gen_S(0), gen_proj(2)])
            run_phase([gen_S(1), gen_AV(0), gen_proj(3)])
            run_phase([gen_S(2), gen_AV(1), gen_proj(4)])
            run_phase([gen_S(3), gen_AV(2), gen_Y(0), gen_proj(5)])
            run_phase([gen_S(4), gen_AV(3), gen_proj(6)])
            run_phase([gen_S(5), gen_AV(4), gen_Y(1), gen_proj(7)])
            run_phase([gen_S(6), gen_AV(5)])
            run_phase([gen_S(7), gen_AV(6), gen_Y(2)])
            run_phase([gen_AV(7), gen_Y(3, delay=6)])
            if debug:
                nc.sync.dma_start(out=dbg["dump_q"][:], in_=qrope[0][:])
                nc.sync.dma_start(out=dbg["dump_k"][:], in_=kT2[:])
                nc.sync.dma_start(out=dbg["dump_vaug"][:], in_=v_aug[0][:])

    try:
        nc.compile()
    finally:
        _bacc_mod.get_activation_tables = _orig_tables
    _CACHE[key] = nc
    return nc


def _host_prep(x, cos, sin, Wq, Wk, Wv, Wo):
    x = np.asarray(x, np.float32)
    xT = np.ascontiguousarray(x.reshape(T, D).T).astype(BF)
    cosT = np.asarray(cos, np.float32).T
    sinT = np.asarray(sin, np.float32).T
    cos2 = np.ascontiguousarray(np.tile(cosT, (2, 1)))          # (128, S) f32
    sin2 = np.ascontiguousarray(np.tile(sinT, (2, 1)))
    # lhsT for qshiftT = A @ qT  ->  arot = A.T (block-diag x2 over heads)
    A = np.zeros((HD, HD), np.float32)
    for d in range(32):
        A[d, d + 32] = -1.0
        A[32 + d, d] = 1.0
    arot = np.kron(np.eye(2, dtype=np.float32), A.T).astype(BF)  # (128,128)
    eye64 = np.eye(64, dtype=np.float32).astype(BF)

    Wq = np.asarray(Wq, np.float32)
    Wk = np.asarray(Wk, np.float32)
    Wv = np.asarray(Wv, np.float32)
    Wo = np.asarray(Wo, np.float32)
    in_maps = []
    for g in range(NC):
        wq_g = np.ascontiguousarray(Wq[:, g * QD:(g + 1) * QD]).astype(BF)
        wkv_g = np.ascontiguousarray(
            np.concatenate([Wk[:, g * HD:(g + 1) * HD], Wv[:, g * HD:(g + 1) * HD]], axis=1)
        ).astype(BF)
        wo_g = np.ascontiguousarray(Wo[g * QD:(g + 1) * QD, :]).astype(BF)
        in_maps.append({
            "xT": xT, "wq": wq_g, "wkv": wkv_g, "wo": wo_g,
            "cos2": cos2, "sin2": sin2, "arot": arot, "eye64": eye64,
        })
    return in_maps


def kernel(x, cos, sin, Wq, Wk, Wv, Wo):
    nc = _build()
    in_maps = _host_prep(x, cos, sin, Wq, Wk, Wv, Wo)
    res = bass_utils.run_bass_kernel_spmd(
        nc, in_maps, core_ids=list(range(NC)), trace=False,
    )
    y = np.zeros((T, D), np.float32)
    for r in res.results:
        y += np.asarray(r["y"], np.float32)
    return y.reshape(B, S, D)
